# revision 9
# baseline (speedup 1.0000x reference)
"""AttFusion (per-pixel self-attention over ragged agent groups) on 8 trn2 cores.

Sharding: data-parallel over the W*H pixel axis (each core gets V/8 pixels;
every pixel's attention is independent). Layout stays "natural" ([C, pixels]
with channels on partitions) the whole way:

  scores   s_m[n] = sum_c x0[c,n] * xm[c,n]    -> DVE elementwise mult +
                                                  PE column-sum matmuls
  softmax  w = exp(s/sqrt(C)) / sum            -> ACT exp, PE row-sum,
                                                  DVE reciprocal, PE broadcast
  context  out[c,n] = sum_m w_m[n] * xm[c,n]   -> DVE mult by broadcast
                                                  weights + PE identity-matmul
                                                  accumulation in PSUM

PE operands must start at 32-aligned partitions, so row placement/selection
goes through small constant selector matrices (in `consts`) instead of AP
partition offsets:
  COLSEL_r  [128, M] ones in column r  -> column-sum lands in PSUM row r
  BLOCKSEL  [L7^2, L7] 1 at (k, k//L7) -> per-row softmax sums in one matmul
  SEL       [L7, L7^2] 1 at (l, l*L7+m)-> broadcast recip row l to rows (l,*)
  ROWSEL_m  [K, 128] ones in row m     -> broadcast w row m to 128 partitions

Only the last group needs the full LxL attention map; other groups only need
row 0 (the reference keeps just the ego row).
"""

import numpy as np
from contextlib import ExitStack

N_CORES = 8
PART = 128
CH = 512  # pixel chunk; [128, 512] fp32 = exactly one PSUM bank

# consts column layout (all matmul lhsT are [K, 128]: fp32r needs col_grp=0xf)
_IDENT = 1
_ONES_BLK = 129
_BAND = 257     # [128, 256], only column _BAND+127 is ones; colsel_r slices it
_SELSUM = 513   # [L7^2, 128]: 1 at (k, k//L7)  -> per-row softmax sums
_SELPAD = 641   # [L7, 128]:   1 at (l, l*L7+m) -> broadcast recip row l
_ROWSEL = 769   # 25 matrices [*, 128], ones in row m, stride 128
_CONST_COLS = 4096

_CACHE = {}


def _make_consts(L7):
    c = np.zeros((PART, _CONST_COLS), np.float32)
    c[:, _IDENT:_IDENT + PART] = np.eye(PART, dtype=np.float32)
    c[:, _ONES_BLK:_ONES_BLK + PART] = 1.0
    c[:, _BAND + 127] = 1.0
    for k in range(L7 * L7):
        c[k, _SELSUM + k // L7] = 1.0
    for l in range(L7):
        c[l, _SELPAD + l * L7:_SELPAD + (l + 1) * L7] = 1.0
    for m in range(25):
        c[m, _ROWSEL + m * PART:_ROWSEL + (m + 1) * PART] = 1.0
    return c


def _build_program(rl, C, V):
    import concourse.tile as tile
    from concourse import bacc, mybir

    f32 = mybir.dt.float32
    f32r = mybir.dt.float32r
    rl = list(rl)
    A = int(sum(rl))
    G = len(rl)
    offsets = [0]
    for L in rl:
        offsets.append(offsets[-1] + int(L))
    L7 = int(rl[-1])
    n_core = V // N_CORES
    assert n_core % CH == 0
    nch = n_core // CH
    assert C % PART == 0
    CC = C // PART
    scale = float(1.0 / np.sqrt(np.float32(C)))

    nc = bacc.Bacc("TRN2", target_bir_lowering=False, debug=False,
                   num_devices=N_CORES)
    x_ap = nc.dram_tensor("x", [A, C, n_core], f32, kind="ExternalInput").ap()
    consts_ap = nc.dram_tensor("consts", [PART, _CONST_COLS], f32,
                               kind="ExternalInput").ap()
    out_ap = nc.dram_tensor("out", [G, C, n_core], f32,
                            kind="ExternalOutput").ap()
    attn_ap = nc.dram_tensor("attn", [L7 * L7, n_core], f32,
                             kind="ExternalOutput").ap()

    with tile.TileContext(nc) as tc, ExitStack() as ctx:
        const_pool = ctx.enter_context(tc.tile_pool(name="consts", bufs=1))
        xpool = ctx.enter_context(tc.tile_pool(name="xp", bufs=min(A + 2, 30)))
        prodp = ctx.enter_context(tc.tile_pool(name="prodp", bufs=4))
        partp = ctx.enter_context(tc.tile_pool(name="partp", bufs=4))
        osbp = ctx.enter_context(tc.tile_pool(name="osbp", bufs=2))
        ewp = ctx.enter_context(tc.tile_pool(name="ewp", bufs=2))
        rp = ctx.enter_context(tc.tile_pool(name="rp", bufs=2))
        psS = ctx.enter_context(tc.tile_pool(name="psS", bufs=2, space="PSUM"))
        psSum = ctx.enter_context(tc.tile_pool(name="psSum", bufs=1, space="PSUM"))
        psBc = ctx.enter_context(tc.tile_pool(name="psBc", bufs=1, space="PSUM"))
        psWb = ctx.enter_context(tc.tile_pool(name="psWb", bufs=2, space="PSUM"))
        psOut = ctx.enter_context(tc.tile_pool(name="psOut", bufs=1, space="PSUM"))

        def R(ap):
            return ap.bitcast(f32r)

        cons = const_pool.tile([PART, _CONST_COLS], f32, tag="consts")
        nc.sync.dma_start(R(cons[:]), consts_ap[:].bitcast(f32r))

        ident = cons[:, _IDENT:_IDENT + PART]

        def colsel(r):
            return cons[:, _BAND + 127 - r:_BAND + 255 - r]

        def rowsel(m, k):
            return cons[0:k, _ROWSEL + m * PART:_ROWSEL + (m + 1) * PART]

        Exp = mybir.ActivationFunctionType.Exp

        for ch in range(nch):
            X = []
            for a in range(A):
                xt = xpool.tile([PART, CC * CH], f32, tag="X")
                src = x_ap[a].rearrange("(cc p) n -> p cc n", p=PART)
                nc.sync.dma_start(
                    xt[:].rearrange("p (cc n) -> p cc n", n=CH),
                    src[:, :, ch * CH:(ch + 1) * CH])
                X.append(xt)

            for g in range(G):
                base, L = offsets[g], int(rl[g])
                last = g == G - 1
                nrows = L * L if last else L
                nsum = L if last else 1

                S = psS.tile([PART, CH], f32, tag="S")
                pairs = ([(i, j) for i in range(L) for j in range(i, L)]
                         if last else [(0, j) for j in range(L)])
                mm = []
                for (i, j) in pairs:
                    prod = prodp.tile([PART, CC * CH], f32, tag="prod")
                    nc.vector.tensor_mul(R(prod[:]), X[base + i][:], X[base + j][:])
                    rows = [i * L + j] if last else [j]
                    if last and i != j:
                        rows.append(j * L + i)
                    for r_ in rows:
                        for cc in range(CC):
                            mm.append((colsel(r_),
                                       prod[:, cc * CH:(cc + 1) * CH]))
                for q, (lhsT, rhs) in enumerate(mm):
                    nc.tensor.matmul(S[:, :], R(lhsT), R(rhs),
                                     start=(q == 0), stop=(q == len(mm) - 1))

                # softmax without max-subtraction (scores are bounded; fp32
                # exp is safe) and normalization deferred: w = e * (1/sum)
                e = ewp.tile([32, CH], f32, tag="e")
                nc.scalar.activation(R(e[:nrows, :]), S[:nrows, :], Exp, scale=scale)
                Ssum = psSum.tile([PART, CH], f32, tag="sum")
                if last:
                    nc.tensor.matmul(Ssum[:, :],
                                     R(cons[0:nrows, _SELSUM:_SELSUM + PART]),
                                     R(e[0:nrows, :]), start=True, stop=True)
                else:
                    nc.tensor.matmul(Ssum[:, :], R(colsel(0)[0:L, :]),
                                     R(e[0:L, :]), start=True, stop=True)
                r = rp.tile([8, CH], f32, tag="r")
                with nc.allow_low_precision(reason="fp32r 11-bit mantissa is ample for softmax"):
                    nc.vector.reciprocal(R(r[:nsum, :]), Ssum[0:nsum, :])
                Bc = psBc.tile([PART, CH], f32, tag="bc")
                if last:
                    nc.tensor.matmul(Bc[:, :],
                                     R(cons[0:nsum, _SELPAD:_SELPAD + PART]),
                                     R(r[0:nsum, :]), start=True, stop=True)
                else:
                    nc.tensor.matmul(Bc[:, :],
                                     R(cons[0:1, _ONES_BLK:_ONES_BLK + PART]),
                                     R(r[0:1, :]), start=True, stop=True)
                w = ewp.tile([32, CH], f32, tag="w")
                nc.vector.tensor_mul(R(w[:nrows, :]), e[:nrows, :], Bc[0:nrows, :])
                if last:
                    nc.sync.dma_start(attn_ap[:, ch * CH:(ch + 1) * CH],
                                      w[:L7 * L7, :])

                # context: out = sum_m w_m (broadcast over C) * X_m
                Wb = []
                for m in range(L):
                    wb = psWb.tile([PART, CH], f32, tag="Wb")
                    nc.tensor.matmul(wb[:], R(rowsel(m, nrows)), R(w[0:nrows, :]),
                                     start=True, stop=True)
                    Wb.append(wb)
                oacc = psOut.tile([PART, CC * CH], f32, tag="OutAcc")
                for m in range(L):
                    part = partp.tile([PART, CC * CH], f32, tag="part")
                    for cc in range(CC):
                        nc.vector.tensor_mul(
                            R(part[:, cc * CH:(cc + 1) * CH]),
                            X[base + m][:, cc * CH:(cc + 1) * CH], Wb[m][:])
                    for cc in range(CC):
                        nc.tensor.matmul(
                            oacc[:, cc * CH:(cc + 1) * CH], R(ident),
                            R(part[:, cc * CH:(cc + 1) * CH]),
                            start=(m == 0), stop=(m == L - 1))
                osb = osbp.tile([PART, CC * CH], f32, tag="osb")
                nc.scalar.copy(osb[:], oacc[:])
                dst = out_ap[g].rearrange("(cc p) n -> p cc n", p=PART)
                nc.sync.dma_start(dst[:, :, ch * CH:(ch + 1) * CH],
                                  osb[:].rearrange("p (cc n) -> p cc n", n=CH))

    nc.compile()
    return nc


def _get_program(rl, C, V):
    key = (tuple(rl), C, V)
    if key not in _CACHE:
        _CACHE[key] = _build_program(rl, C, V)
    return _CACHE[key]


def kernel(x, record_len):
    from concourse.bass_utils import run_bass_kernel_spmd

    x = np.asarray(x, dtype=np.float32)
    rl = [int(v) for v in np.asarray(record_len)]
    A, C, W, H = x.shape
    assert A == sum(rl)
    V = W * H
    n_core = V // N_CORES
    G = len(rl)
    L7 = rl[-1]

    nc = _get_program(rl, C, V)
    consts = _make_consts(L7)
    xf = x.reshape(A, C, V)
    in_maps = [
        {"x": np.ascontiguousarray(xf[:, :, k * n_core:(k + 1) * n_core]),
         "consts": consts}
        for k in range(N_CORES)
    ]
    res = run_bass_kernel_spmd(nc, in_maps, list(range(N_CORES))).results
    out = np.concatenate([res[k]["out"] for k in range(N_CORES)], axis=2)
    attn = np.concatenate([res[k]["attn"] for k in range(N_CORES)], axis=1)
    return (out.reshape(G, C, W, H), attn.reshape(L7, L7, W, H))


# revision 16
# speedup vs baseline: 1.2190x; 1.2190x over previous
"""AttFusion (per-pixel self-attention over ragged agent groups) on 8 trn2 cores.

Sharding: data-parallel over the W*H pixel axis (each core gets V/8 pixels;
every pixel's attention is independent). Layout stays "natural" ([C, pixels]
with channels on partitions) the whole way:

  scores   s_m[n] = sum_c x0[c,n] * xm[c,n]    -> DVE elementwise mult +
                                                  PE column-sum matmuls
  softmax  w = exp(s/sqrt(C)) / sum            -> ACT exp, PE row-sum,
                                                  DVE reciprocal, PE broadcast
  context  out[c,n] = sum_m w_m[n] * xm[c,n]   -> DVE mult by broadcast
                                                  weights + PE identity-matmul
                                                  accumulation in PSUM

PE operands must start at 32-aligned partitions, so row placement/selection
goes through small constant selector matrices (in `consts`) instead of AP
partition offsets:
  COLSEL_r  [128, M] ones in column r  -> column-sum lands in PSUM row r
  BLOCKSEL  [L7^2, L7] 1 at (k, k//L7) -> per-row softmax sums in one matmul
  SEL       [L7, L7^2] 1 at (l, l*L7+m)-> broadcast recip row l to rows (l,*)
  ROWSEL_m  [K, 128] ones in row m     -> broadcast w row m to 128 partitions

Only the last group needs the full LxL attention map; other groups only need
row 0 (the reference keeps just the ego row).
"""

import numpy as np
from contextlib import ExitStack

N_CORES = 8
PART = 128
CH = 512  # pixel chunk; [128, 512] fp32 = exactly one PSUM bank

# consts column layout (all matmul lhsT are [K, 128]: fp32r needs col_grp=0xf)
_IDENT = 1
_ONES_BLK = 129
_BAND = 257     # [128, 256], only column _BAND+127 is ones; colsel_r slices it
_SELSUM = 513   # [L7^2, 128]: 1 at (k, k//L7)  -> per-row softmax sums
_SELPAD = 641   # [L7, L7^2]:  1 at (l, l*L7+m) -> broadcast recip row l
_CONST_COLS = 768

_CACHE = {}


def _make_consts(L7):
    c = np.zeros((PART, _CONST_COLS), np.float32)
    c[:, _IDENT:_IDENT + PART] = np.eye(PART, dtype=np.float32)
    c[:, _ONES_BLK:_ONES_BLK + PART] = 1.0
    c[:, _BAND + 127] = 1.0
    for k in range(L7 * L7):
        c[k, _SELSUM + k // L7] = 1.0
    for l in range(L7):
        c[l, _SELPAD + l * L7:_SELPAD + (l + 1) * L7] = 1.0
    return c


def _build_program(rl, C, V):
    import concourse.tile as tile
    from concourse import bacc, mybir

    f32 = mybir.dt.float32
    f32r = mybir.dt.float32r
    rl = list(rl)
    A = int(sum(rl))
    G = len(rl)
    offsets = [0]
    for L in rl:
        offsets.append(offsets[-1] + int(L))
    L7 = int(rl[-1])
    n_core = V // N_CORES
    assert n_core % CH == 0
    nch = n_core // CH
    assert C % PART == 0
    CC = C // PART
    scale = float(1.0 / np.sqrt(np.float32(C)))

    nc = bacc.Bacc("TRN2", target_bir_lowering=False, debug=False,
                   num_devices=N_CORES)
    x_ap = nc.dram_tensor("x", [A, C, n_core], f32, kind="ExternalInput").ap()
    consts_ap = nc.dram_tensor("consts", [PART, _CONST_COLS], f32,
                               kind="ExternalInput").ap()
    out_ap = nc.dram_tensor("out", [G, C, n_core], f32,
                            kind="ExternalOutput").ap()
    attn_ap = nc.dram_tensor("attn", [L7 * L7, n_core], f32,
                             kind="ExternalOutput").ap()

    with tile.TileContext(nc) as tc, ExitStack() as ctx:
        const_pool = ctx.enter_context(tc.tile_pool(name="consts", bufs=1))
        xpool = ctx.enter_context(tc.tile_pool(name="xp", bufs=min(A + 2, 30)))
        prodp = ctx.enter_context(tc.tile_pool(name="prodp", bufs=5))
        partp = ctx.enter_context(tc.tile_pool(name="partp", bufs=4))
        osbp = ctx.enter_context(tc.tile_pool(name="osbp", bufs=2))
        wbsp = ctx.enter_context(tc.tile_pool(name="wbsp", bufs=3))
        ewp = ctx.enter_context(tc.tile_pool(name="ewp", bufs=2))
        rp = ctx.enter_context(tc.tile_pool(name="rp", bufs=2))
        psS = ctx.enter_context(tc.tile_pool(name="psS", bufs=2, space="PSUM"))
        psSum = ctx.enter_context(tc.tile_pool(name="psSum", bufs=1, space="PSUM"))
        psBc = ctx.enter_context(tc.tile_pool(name="psBc", bufs=1, space="PSUM"))
        psWb = ctx.enter_context(tc.tile_pool(name="psWb", bufs=2, space="PSUM"))
        psOut = ctx.enter_context(tc.tile_pool(name="psOut", bufs=1, space="PSUM"))

        def R(ap):
            return ap.bitcast(f32r)

        cons = const_pool.tile([PART, _CONST_COLS], f32, tag="consts")
        nc.sync.dma_start(R(cons[:]), consts_ap[:].bitcast(f32r))

        ident = cons[:, _IDENT:_IDENT + PART]

        def colsel(r):
            return cons[:, _BAND + 127 - r:_BAND + 255 - r]

        def rowsel(m, k):
            # ones in row m of a [k, 128] matrix == identity column m
            # repeated 128 times (step-0 free dim)
            return ident[0:k, m:m + 1].broadcast_to((k, PART))

        Exp = mybir.ActivationFunctionType.Exp

        for ch in range(nch):
            X = []
            for a in range(A):
                xt = xpool.tile([PART, CC * CH], f32, tag="X")
                src = x_ap[a].rearrange("(cc p) n -> p cc n", p=PART)
                nc.sync.dma_start(
                    xt[:].rearrange("p (cc n) -> p cc n", n=CH),
                    src[:, :, ch * CH:(ch + 1) * CH])
                X.append(xt)

            for g in range(G):
                base, L = offsets[g], int(rl[g])
                last = g == G - 1
                nrows = L * L if last else L
                nsum = L if last else 1

                S = psS.tile([PART, CH], f32, tag="S")
                pairs = ([(i, j) for i in range(L) for j in range(i, L)]
                         if last else [(0, j) for j in range(L)])
                mm = []
                for (i, j) in pairs:
                    prod = prodp.tile([PART, CC * CH], f32, tag="prod")
                    if i == j:
                        # self-products on the (otherwise idle) scalar engine
                        nc.scalar.square(R(prod[:]), X[base + i][:])
                    else:
                        nc.vector.tensor_mul(R(prod[:]), X[base + i][:], X[base + j][:])
                    rows = [i * L + j] if last else [j]
                    if last and i != j:
                        rows.append(j * L + i)
                    for r_ in rows:
                        for cc in range(CC):
                            mm.append((colsel(r_),
                                       prod[:, cc * CH:(cc + 1) * CH]))
                for q, (lhsT, rhs) in enumerate(mm):
                    nc.tensor.matmul(S[:, :], R(lhsT), R(rhs),
                                     start=(q == 0), stop=(q == len(mm) - 1))

                # softmax without max-subtraction (scores are bounded; fp32
                # exp is safe) and normalization deferred: w = e * (1/sum)
                e = ewp.tile([32, CH], f32, tag="e")
                nc.scalar.activation(R(e[:nrows, :]), S[:nrows, :], Exp, scale=scale)
                Ssum = psSum.tile([PART, CH], f32, tag="sum")
                if last:
                    nc.tensor.matmul(Ssum[:, :],
                                     R(cons[0:nrows, _SELSUM:_SELSUM + PART]),
                                     R(e[0:nrows, :]), start=True, stop=True)
                else:
                    nc.tensor.matmul(Ssum[:, :], R(colsel(0)[0:L, :]),
                                     R(e[0:L, :]), start=True, stop=True)
                # reciprocal_approx_fast streams at line rate (real HW
                # Reciprocal is an 8-cyc/elem iterative divide). The broadcast
                # matmul stays plain fp32 (4 cyc/row, PE has slack) so the
                # unrounded reciprocal can feed it directly.
                r = rp.tile([8, CH], f32, tag="r")
                nc.vector.reciprocal_approx_fast(r[:nsum, :], Ssum[0:nsum, :])
                Bc = psBc.tile([PART, CH], f32, tag="bc")
                if last:
                    nc.tensor.matmul(Bc[0:nrows, :],
                                     cons[0:nsum, _SELPAD:_SELPAD + nrows],
                                     r[0:nsum, :], start=True, stop=True)
                else:
                    nc.tensor.matmul(Bc[:, :],
                                     cons[0:1, _ONES_BLK:_ONES_BLK + PART],
                                     r[0:1, :], start=True, stop=True)
                w = ewp.tile([32, CH], f32, tag="w")
                nc.vector.tensor_mul(R(w[:nrows, :]), e[:nrows, :], Bc[0:nrows, :])
                if last:
                    nc.sync.dma_start(attn_ap[:, ch * CH:(ch + 1) * CH],
                                      w[:L7 * L7, :])

                # context: out = sum_m w_m (broadcast over C) * X_m
                Wb = []
                for m in range(L):
                    wb = psWb.tile([PART, CH], f32, tag="Wb")
                    nc.tensor.matmul(wb[:], R(rowsel(m, nrows)), R(w[0:nrows, :]),
                                     start=True, stop=True)
                    Wb.append(wb)
                oacc = psOut.tile([PART, CC * CH], f32, tag="OutAcc")
                for m in range(L):
                    wbs = wbsp.tile([PART, CH], f32, tag="wbs")
                    nc.scalar.copy(wbs[:], Wb[m][:])
                    part = partp.tile([PART, CC * CH], f32, tag="part")
                    nc.vector.tensor_mul(
                        R(part[:].rearrange("p (cc n) -> p cc n", n=CH)),
                        X[base + m][:].rearrange("p (cc n) -> p cc n", n=CH),
                        wbs[:].unsqueeze(1).broadcast_to((PART, CC, CH)))
                    for cc in range(CC):
                        nc.tensor.matmul(
                            oacc[:, cc * CH:(cc + 1) * CH], R(ident),
                            R(part[:, cc * CH:(cc + 1) * CH]),
                            start=(m == 0), stop=(m == L - 1))
                osb = osbp.tile([PART, CC * CH], f32, tag="osb")
                nc.scalar.copy(osb[:], oacc[:])
                dst = out_ap[g].rearrange("(cc p) n -> p cc n", p=PART)
                nc.sync.dma_start(dst[:, :, ch * CH:(ch + 1) * CH],
                                  osb[:].rearrange("p (cc n) -> p cc n", n=CH))

    nc.compile()
    return nc


def _get_program(rl, C, V):
    key = (tuple(rl), C, V)
    if key not in _CACHE:
        _CACHE[key] = _build_program(rl, C, V)
    return _CACHE[key]


def kernel(x, record_len):
    from concourse.bass_utils import run_bass_kernel_spmd

    x = np.asarray(x, dtype=np.float32)
    rl = [int(v) for v in np.asarray(record_len)]
    A, C, W, H = x.shape
    assert A == sum(rl)
    V = W * H
    n_core = V // N_CORES
    G = len(rl)
    L7 = rl[-1]

    nc = _get_program(rl, C, V)
    consts = _make_consts(L7)
    xf = x.reshape(A, C, V)
    in_maps = [
        {"x": np.ascontiguousarray(xf[:, :, k * n_core:(k + 1) * n_core]),
         "consts": consts}
        for k in range(N_CORES)
    ]
    res = run_bass_kernel_spmd(nc, in_maps, list(range(N_CORES))).results
    out = np.concatenate([res[k]["out"] for k in range(N_CORES)], axis=2)
    attn = np.concatenate([res[k]["attn"] for k in range(N_CORES)], axis=1)
    return (out.reshape(G, C, W, H), attn.reshape(L7, L7, W, H))
